# revision 16
# baseline (speedup 1.0000x reference)
"""Single-head causal attention (B=8, T=2048, C=1024, H=64) on 8 TRN2 NeuronCores.

Strategy (data-parallel over batch, one batch element per core):
  - Host transposes x[b] -> xT [C, T] and casts matmul operands to bf16.
  - Device, per core, pipelined per 512-wide t-block tb:
      proj(tb):  qT,kT = ([Wq|Wk].T @ xT_tb) packed in one PE pass; vT = Wv.T @ xT_tb
      evac(tb):  PSUM -> SBUF bf16 casts (kT via 64->0 partition-shift DVE copy)
      trans(tb): v chunks rebuilt in natural [s, h] layout via PE transpose,
                 with a ones-column block appended (v_ext) so the PV matmul
                 also produces the softmax denominator l for free.
      attn(tb), per s-chunk pair (causally trimmed):
          ST[s, t] = kT_chunk.T @ qT_block            (PSUM, 2 banks/pair)
          diag chunks: += causal additive mask on first 128 cols (DVE)
          PT = exp(SCALE * ST)                        (one ACT per pair, bf16 out)
          PV[:, t] += v_ext_chunk.T @ PT              (rows 0-63 = out.T, 64-127 = l)
          out.T = PV[0:64] * reciprocal_approx(PV[64:128])
  - Host transposes outT [H, T] back to [T, H].
All matmul accumulation is fp32 (PSUM); bf16 operands give ~3.4e-3 l2 rel err.
"""

import numpy as np
import ml_dtypes
from contextlib import ExitStack

import concourse.bass as bass
from concourse import bacc
import concourse.mybir as mybir
import concourse.tile as tile
from concourse.bass import ts
from concourse.bass_utils import run_bass_kernel_spmd


B, T, C, H = 8, 2048, 1024, 64
P = 128
W_BLK = 512
N_TB = T // W_BLK       # 4 t-blocks
N_C = C // P            # 8 contraction chunks
N_S = T // P            # 16 s-chunks
N_J = W_BLK // P        # 4 diagonal chunks per t-block
SCALE = float(H) ** -0.5
NEG = -1e30

MM_DT = mybir.dt.bfloat16
NP_MM = ml_dtypes.bfloat16
F32 = mybir.dt.float32


def build_nc() -> bacc.Bacc:
    nc = bacc.Bacc("TRN2")
    xT_d = nc.dram_tensor("xT", [C, T], MM_DT, kind="ExternalInput")
    wqk_d = nc.dram_tensor("Wqk", [C, 2 * H], MM_DT, kind="ExternalInput")
    wv_d = nc.dram_tensor("Wv", [C, H], MM_DT, kind="ExternalInput")
    ident_d = nc.dram_tensor("ident", [H, H], MM_DT, kind="ExternalInput")
    cmask_d = nc.dram_tensor("cmask", [P, P], F32, kind="ExternalInput")
    outT_d = nc.dram_tensor("outT", [H, T], F32, kind="ExternalOutput")

    with tile.TileContext(nc) as tc, ExitStack() as ctx:
        const = ctx.enter_context(tc.tile_pool(name="const", bufs=1))

        wqk_sb = const.tile([P, N_C, 2 * H], MM_DT)
        nc.sync.dma_start(wqk_sb, wqk_d[:].rearrange("(o p) m -> p o m", p=P))
        wv_sb = const.tile([P, N_C, H], MM_DT)
        nc.sync.dma_start(wv_sb, wv_d[:].rearrange("(o p) m -> p o m", p=P))
        ident = const.tile([H, H], MM_DT)
        nc.sync.dma_start(ident, ident_d[:])
        cmask = const.tile([P, P], F32)
        nc.sync.dma_start(cmask, cmask_d[:])

        # xT streamed as independent [128, 512] tiles so each proj matmul only
        # waits on its own DMA (t-block-major order feeds the pipeline head).
        xt = {}
        for tb in range(N_TB):
            for c in range(N_C):
                t_ = const.tile([P, W_BLK], MM_DT, name=f"xt{c}_{tb}")
                eng = nc.sync if c % 2 == 0 else nc.gpsimd
                eng.dma_start(t_, xT_d[ts(c, P), ts(tb, W_BLK)])
                xt[(c, tb)] = t_

        qT_blk = [const.tile([H, W_BLK], MM_DT, name=f"qT{tb}") for tb in range(N_TB)]
        kT_blk = [const.tile([H, W_BLK], MM_DT, name=f"kT{tb}") for tb in range(N_TB)]
        vT_blk = [const.tile([H, W_BLK], MM_DT, name=f"vT{tb}") for tb in range(N_TB)]
        vext = [const.tile([P, P], MM_DT, name=f"vext{s}") for s in range(N_S)]
        for s in range(N_S):
            nc.vector.memset(vext[s][:, H:P], 1.0)

        with tc.tile_pool(name="ps_qk", bufs=2, space="PSUM") as ps_qk, \
             tc.tile_pool(name="ps_v", bufs=1, space="PSUM") as ps_v, \
             tc.tile_pool(name="ps_st", bufs=4, space="PSUM") as ps_st, \
             tc.tile_pool(name="ps_pv", bufs=1, space="PSUM") as ps_pv, \
             tc.tile_pool(name="ptp", bufs=6) as pt_pool, \
             tc.tile_pool(name="outp", bufs=4) as out_pool:

            for tb in range(N_TB):
                # ---- proj(tb) ----
                qk_ps = ps_qk.tile([P, W_BLK], F32, tag="qk", name=f"qk{tb}")
                v_ps = ps_v.tile([H, W_BLK], F32, tag="v", name=f"v{tb}")
                for c in range(N_C):
                    nc.tensor.matmul(qk_ps, wqk_sb[:, c, :], xt[(c, tb)],
                                     start=(c == 0), stop=(c == N_C - 1))
                for c in range(N_C):
                    nc.tensor.matmul(v_ps, wv_sb[:, c, :], xt[(c, tb)],
                                     start=(c == 0), stop=(c == N_C - 1))
                nc.vector.tensor_copy(qT_blk[tb][:], qk_ps[0:H, :])
                # partition shift 64->0 (64-lane DVE op, quadrant-aligned)
                nc.vector.tensor_copy(kT_blk[tb][:], qk_ps[H:P, :])
                nc.vector.tensor_copy(vT_blk[tb][:], v_ps[:, :])

                # ---- v transposes for this block (shares the qk psum tag) ----
                for j in range(N_J):
                    s = tb * N_J + j
                    tr = ps_qk.tile([P, H], MM_DT, tag="qk", name=f"tr{s}")
                    nc.tensor.transpose(tr, vT_blk[tb][:, ts(j, P)], ident)
                    nc.vector.tensor_copy(vext[s][:, 0:H], tr)

                # ---- attn(tb) ----
                t0 = tb * W_BLK
                pv = ps_pv.tile([P, W_BLK], F32, tag="pv", name=f"pv{tb}")
                n_full = tb * N_J
                # (s_chunk, col offset within t-block, width)
                chunks = [(s, 0, W_BLK) for s in range(n_full)]
                chunks += [(n_full + j, j * P, W_BLK - j * P) for j in range(N_J)]
                n_ch = len(chunks)
                for ci, (s, off, w) in enumerate(chunks):
                    st_t = ps_st.tile([P, W_BLK], F32, tag="st",
                                      name=f"st{tb}_{ci}")
                    nc.tensor.matmul(st_t[:, 0:w], kT_blk[s // N_J][:, ts(s % N_J, P)],
                                     qT_blk[tb][:, off:W_BLK],
                                     start=True, stop=True)
                    if s >= n_full:  # diagonal: mask first 128 cols
                        nc.vector.tensor_tensor(st_t[:, 0:P], st_t[:, 0:P],
                                                cmask, mybir.AluOpType.add)
                    ptile = pt_pool.tile([P, W_BLK], MM_DT, tag="pt",
                                         name=f"pt{tb}_{ci}")
                    nc.scalar.activation(ptile[:, 0:w], st_t[:, 0:w],
                                         mybir.ActivationFunctionType.Exp,
                                         scale=SCALE)
                    nc.tensor.matmul(pv[:, off:W_BLK], vext[s],
                                     ptile[:, 0:w],
                                     start=(ci == 0),
                                     stop=(ci == n_ch - 1))
                lsb = out_pool.tile([H, W_BLK], F32, tag="lsb", name=f"lsb{tb}")
                nc.vector.tensor_copy(lsb, pv[H:P, :])
                rl = out_pool.tile([H, W_BLK], F32, tag="rl", name=f"rl{tb}")
                nc.vector.reciprocal_approx_fast(out=rl, in_=lsb)
                ot = out_pool.tile([H, W_BLK], F32, tag="ot", name=f"ot{tb}")
                nc.vector.tensor_tensor(ot, pv[0:H, :], rl, mybir.AluOpType.mult)
                nc.sync.dma_start(outT_d[:, ts(tb, W_BLK)], ot)

    nc.compile()
    return nc


_NC_CACHE = None


def _get_nc():
    global _NC_CACHE
    if _NC_CACHE is None:
        _NC_CACHE = build_nc()
    return _NC_CACHE


def prepare_in_maps(x, Wk, Wq, Wv):
    wqk = np.concatenate([np.asarray(Wq), np.asarray(Wk)], axis=1).astype(NP_MM)
    wv = np.asarray(Wv).astype(NP_MM)
    ident = np.eye(H, dtype=NP_MM)
    # cmask[s, t] = 0 if t >= s else NEG (additive causal mask for diag chunks)
    ii = np.arange(P)
    cmask = np.where(ii[None, :] >= ii[:, None], 0.0, NEG).astype(np.float32)
    in_maps = []
    for b in range(B):
        xT = np.ascontiguousarray(np.asarray(x[b]).T).astype(NP_MM)
        in_maps.append(
            {"xT": xT, "Wqk": wqk, "Wv": wv, "ident": ident, "cmask": cmask}
        )
    return in_maps


def run(x, Wk, Wq, Wv, trace=False):
    nc = _get_nc()
    in_maps = prepare_in_maps(x, Wk, Wq, Wv)
    res = run_bass_kernel_spmd(nc, in_maps, core_ids=list(range(B)), trace=trace)
    out = np.stack([np.asarray(r["outT"], dtype=np.float32).T for r in res.results])
    return out, res


def kernel(x, Wk, Wq, Wv):
    out, _ = run(x, Wk, Wq, Wv, trace=False)
    return out


# revision 17
# speedup vs baseline: 1.0783x; 1.0783x over previous
"""Single-head causal attention (B=8, T=2048, C=1024, H=64) on 8 TRN2 NeuronCores.

Strategy (data-parallel over batch, one batch element per core):
  - Host transposes x[b] -> xT [C, T] and casts matmul operands to bf16.
  - Device, per core, pipelined per 512-wide t-block tb:
      proj(tb):  qT,kT = ([Wq|Wk].T @ xT_tb) packed in one PE pass; vT = Wv.T @ xT_tb
      evac(tb):  PSUM -> SBUF bf16 casts (kT via 64->0 partition-shift DVE copy)
      trans(tb): v chunks rebuilt in natural [s, h] layout via PE transpose,
                 with a ones-column block appended (v_ext) so the PV matmul
                 also produces the softmax denominator l for free.
      attn(tb), per s-chunk pair (causally trimmed):
          ST[s, t] = kT_chunk.T @ qT_block            (PSUM, 2 banks/pair)
          diag chunks: += causal additive mask on first 128 cols (DVE)
          PT = exp(SCALE * ST)                        (one ACT per pair, bf16 out)
          PV[:, t] += v_ext_chunk.T @ PT              (rows 0-63 = out.T, 64-127 = l)
          out.T = PV[0:64] * reciprocal_approx(PV[64:128])
  - Host transposes outT [H, T] back to [T, H].
All matmul accumulation is fp32 (PSUM); bf16 operands give ~3.4e-3 l2 rel err.
"""

import numpy as np
import ml_dtypes
from contextlib import ExitStack

import concourse.bass as bass
from concourse import bacc
import concourse.mybir as mybir
import concourse.tile as tile
from concourse.bass import ts
from concourse.bass_utils import run_bass_kernel_spmd


B, T, C, H = 8, 2048, 1024, 64
P = 128
W_BLK = 512
N_TB = T // W_BLK       # 4 t-blocks
N_C = C // P            # 8 contraction chunks
N_S = T // P            # 16 s-chunks
N_J = W_BLK // P        # 4 diagonal chunks per t-block
SCALE = float(H) ** -0.5
NEG = -1e30

MM_DT = mybir.dt.bfloat16
NP_MM = ml_dtypes.bfloat16
F32 = mybir.dt.float32


def build_nc() -> bacc.Bacc:
    nc = bacc.Bacc("TRN2")
    xT_d = nc.dram_tensor("xT", [C, T], MM_DT, kind="ExternalInput")
    wqk_d = nc.dram_tensor("Wqk", [C, 2 * H], MM_DT, kind="ExternalInput")
    wv_d = nc.dram_tensor("Wv", [C, H], MM_DT, kind="ExternalInput")
    ident_d = nc.dram_tensor("ident", [H, H], MM_DT, kind="ExternalInput")
    cmask_d = nc.dram_tensor("cmask", [P, P], F32, kind="ExternalInput")
    outT_d = nc.dram_tensor("outT", [H, T], F32, kind="ExternalOutput")

    with tile.TileContext(nc) as tc, ExitStack() as ctx:
        const = ctx.enter_context(tc.tile_pool(name="const", bufs=1))

        wqk_sb = const.tile([P, N_C, 2 * H], MM_DT)
        nc.sync.dma_start(wqk_sb, wqk_d[:].rearrange("(o p) m -> p o m", p=P))
        wv_sb = const.tile([P, N_C, H], MM_DT)
        nc.sync.dma_start(wv_sb, wv_d[:].rearrange("(o p) m -> p o m", p=P))
        ident = const.tile([H, H], MM_DT)
        nc.sync.dma_start(ident, ident_d[:])
        cmask = const.tile([P, P], F32)
        nc.sync.dma_start(cmask, cmask_d[:])

        # xT streamed as independent [128, 512] tiles so each proj matmul only
        # waits on its own DMA (t-block-major order feeds the pipeline head).
        xt = {}
        for tb in range(N_TB):
            for c in range(N_C):
                t_ = const.tile([P, W_BLK], MM_DT, name=f"xt{c}_{tb}")
                nc.sync.dma_start(t_, xT_d[ts(c, P), ts(tb, W_BLK)])
                xt[(c, tb)] = t_

        qT_blk = [const.tile([H, W_BLK], MM_DT, name=f"qT{tb}") for tb in range(N_TB)]
        kT_blk = [const.tile([H, W_BLK], MM_DT, name=f"kT{tb}") for tb in range(N_TB)]
        vT_blk = [const.tile([H, W_BLK], MM_DT, name=f"vT{tb}") for tb in range(N_TB)]
        vext = [const.tile([P, P], MM_DT, name=f"vext{s}") for s in range(N_S)]
        for s in range(N_S):
            nc.vector.memset(vext[s][:, H:P], 1.0)

        with tc.tile_pool(name="ps_qk", bufs=1, space="PSUM") as ps_qk, \
             tc.tile_pool(name="ps_v", bufs=1, space="PSUM") as ps_v, \
             tc.tile_pool(name="ps_st", bufs=4, space="PSUM") as ps_st, \
             tc.tile_pool(name="ps_pv", bufs=2, space="PSUM") as ps_pv, \
             tc.tile_pool(name="ptp", bufs=8) as pt_pool, \
             tc.tile_pool(name="outp", bufs=6) as out_pool:

            for tb in range(N_TB):
                # ---- proj(tb) ----
                qk_ps = ps_qk.tile([P, W_BLK], F32, tag="qk", name=f"qk{tb}")
                v_ps = ps_v.tile([H, W_BLK], F32, tag="v", name=f"v{tb}")
                for c in range(N_C):
                    nc.tensor.matmul(qk_ps, wqk_sb[:, c, :], xt[(c, tb)],
                                     start=(c == 0), stop=(c == N_C - 1))
                for c in range(N_C):
                    nc.tensor.matmul(v_ps, wv_sb[:, c, :], xt[(c, tb)],
                                     start=(c == 0), stop=(c == N_C - 1))
                nc.vector.tensor_copy(qT_blk[tb][:], qk_ps[0:H, :])
                # partition shift 64->0 (64-lane DVE op, quadrant-aligned)
                nc.vector.tensor_copy(kT_blk[tb][:], qk_ps[H:P, :])
                nc.vector.tensor_copy(vT_blk[tb][:], v_ps[:, :])

                # ---- v transposes for this block (shares the qk psum tag) ----
                for j in range(N_J):
                    s = tb * N_J + j
                    tr = ps_qk.tile([P, H], MM_DT, tag="qk", name=f"tr{s}")
                    nc.tensor.transpose(tr, vT_blk[tb][:, ts(j, P)], ident)
                    nc.vector.tensor_copy(vext[s][:, 0:H], tr)

                # ---- attn(tb) ----
                t0 = tb * W_BLK
                pv = ps_pv.tile([P, W_BLK], F32, tag="pv", name=f"pv{tb}")
                n_full = tb * N_J
                # (s_chunk, col offset within t-block, width)
                chunks = [(s, 0, W_BLK) for s in range(n_full)]
                chunks += [(n_full + j, j * P, W_BLK - j * P) for j in range(N_J)]
                n_ch = len(chunks)
                for ci, (s, off, w) in enumerate(chunks):
                    st_t = ps_st.tile([P, W_BLK], F32, tag="st",
                                      name=f"st{tb}_{ci}")
                    nc.tensor.matmul(st_t[:, 0:w], kT_blk[s // N_J][:, ts(s % N_J, P)],
                                     qT_blk[tb][:, off:W_BLK],
                                     start=True, stop=True)
                    if s >= n_full:  # diagonal: mask first 128 cols
                        nc.vector.tensor_tensor(st_t[:, 0:P], st_t[:, 0:P],
                                                cmask, mybir.AluOpType.add)
                    ptile = pt_pool.tile([P, W_BLK], MM_DT, tag="pt",
                                         name=f"pt{tb}_{ci}")
                    nc.scalar.activation(ptile[:, 0:w], st_t[:, 0:w],
                                         mybir.ActivationFunctionType.Exp,
                                         scale=SCALE)
                    nc.tensor.matmul(pv[:, off:W_BLK], vext[s],
                                     ptile[:, 0:w],
                                     start=(ci == 0),
                                     stop=(ci == n_ch - 1))
                lsb = out_pool.tile([H, W_BLK], F32, tag="lsb", name=f"lsb{tb}")
                nc.vector.tensor_copy(lsb, pv[H:P, :])
                rl = out_pool.tile([H, W_BLK], F32, tag="rl", name=f"rl{tb}")
                nc.vector.reciprocal_approx_fast(out=rl, in_=lsb)
                ot = out_pool.tile([H, W_BLK], F32, tag="ot", name=f"ot{tb}")
                nc.vector.tensor_tensor(ot, pv[0:H, :], rl, mybir.AluOpType.mult)
                nc.sync.dma_start(outT_d[:, ts(tb, W_BLK)], ot)

    nc.compile()
    return nc


_NC_CACHE = None


def _get_nc():
    global _NC_CACHE
    if _NC_CACHE is None:
        _NC_CACHE = build_nc()
    return _NC_CACHE


def prepare_in_maps(x, Wk, Wq, Wv):
    wqk = np.concatenate([np.asarray(Wq), np.asarray(Wk)], axis=1).astype(NP_MM)
    wv = np.asarray(Wv).astype(NP_MM)
    ident = np.eye(H, dtype=NP_MM)
    # cmask[s, t] = 0 if t >= s else NEG (additive causal mask for diag chunks)
    ii = np.arange(P)
    cmask = np.where(ii[None, :] >= ii[:, None], 0.0, NEG).astype(np.float32)
    in_maps = []
    for b in range(B):
        xT = np.ascontiguousarray(np.asarray(x[b]).T).astype(NP_MM)
        in_maps.append(
            {"xT": xT, "Wqk": wqk, "Wv": wv, "ident": ident, "cmask": cmask}
        )
    return in_maps


def run(x, Wk, Wq, Wv, trace=False):
    nc = _get_nc()
    in_maps = prepare_in_maps(x, Wk, Wq, Wv)
    res = run_bass_kernel_spmd(nc, in_maps, core_ids=list(range(B)), trace=trace)
    out = np.stack([np.asarray(r["outT"], dtype=np.float32).T for r in res.results])
    return out, res


def kernel(x, Wk, Wq, Wv):
    out, _ = run(x, Wk, Wq, Wv, trace=False)
    return out
